# revision 1
# baseline (speedup 1.0000x reference)
"""Trainium2 Bass kernel for nn_MeshUnpool (batched features @ (unroll/occ) matmul).

Reference: out[b] = features[b] @ (unroll_mat[b] / occurrences[b][None, :])
  features:    [4, 256, 4560]  f32
  unroll_mat:  [4, 4560, 9120] f32 (binary 0/1 group-membership)
  occurrences: [4, 9120]       f32 (positive integer counts)
  out:         [4, 256, 9120]  f32

Sharding (8 cores): core c = (b, half) = divmod(c, 2) computes
  out[b, :, half*4560:(half+1)*4560] = features[b] @ unroll[b][:, half] * inv_occ
i.e. batch (4-way) x target-column halves (2-way). This reads each unroll_mat
element exactly once -- the traffic-minimal split.

Per-core kernel: PE matmul with fp16 weights (features^T, host-cast) against
an fp8e4 moving operand (unroll columns, host-cast -- binary 0/1 is EXACT in
fp8e4, so no accuracy loss beyond the fp16 rounding of features, ~2e-4
absmax-relative). Accumulate over 36 K-chunks of 128 edges in PSUM, then
multiply by host-precomputed broadcast 1/occ on the Vector engine during
PSUM->SBUF copyback, and DMA out.
"""
import numpy as np
import ml_dtypes

import concourse.bacc as bacc
import concourse.mybir as mybir
from concourse.bass_utils import run_bass_kernel_spmd
from concourse.tile import TileContext

dt = mybir.dt

B, NF, EDGES, TARGET = 4, 256, 4560, 9120
NCORES = 8
COLS = TARGET // 2            # 4560 target columns per core
KCH = (EDGES + 127) // 128    # 36 contraction chunks (35x128 + 80)
SUB = 512                     # matmul moving free dim (one PSUM bank)
GROUP = 1024                  # target columns per PSUM group
GROUPS = [(g0, min(GROUP, COLS - g0)) for g0 in range(0, COLS, GROUP)]

_CACHE = {}
_last_results = None


def _build(reps=1):
    nc = bacc.Bacc("TRN2", target_bir_lowering=False, debug=False)
    fT = nc.declare_dram_parameter("fT", [EDGES, NF], dt.float16, isOutput=False)
    um = nc.declare_dram_parameter("um", [EDGES, COLS], dt.float8e4, isOutput=False)
    inv = nc.declare_dram_parameter("inv", [128, COLS], dt.float32, isOutput=False)
    out = nc.declare_dram_parameter("out", [NF, COLS], dt.float32, isOutput=True)

    with TileContext(nc) as tc:
        with (
            tc.tile_pool(name="ftp", bufs=1) as ftp,
            tc.tile_pool(name="ivp", bufs=1) as ivp,
            tc.tile_pool(name="ump", bufs=28) as ump,
            tc.tile_pool(name="psp", bufs=8, space="PSUM") as psp,
            tc.tile_pool(name="obp", bufs=12) as obp,
        ):
            # Features^T resident in SBUF: 36 chunks of [<=128 edges, 256 nf].
            ft_tiles = []
            for k in range(KCH):
                kp = min(128, EDGES - k * 128)
                t = ftp.tile([128, NF], dt.float16, name=f"ft{k}", tag=f"ft{k}")
                nc.sync.dma_start(t[:kp, :], fT[k * 128:k * 128 + kp, :])
                ft_tiles.append(t)
            # 1/occ broadcast across partitions, resident.
            inv_sb = ivp.tile([128, COLS], dt.float32, name="inv_sb")
            nc.sync.dma_start(inv_sb[:, :], inv[:, :])

            def body():
                for g0, gw in GROUPS:
                    nsubs = [(n0, min(SUB, gw - n0)) for n0 in range(0, gw, SUB)]
                    ps = {}
                    for m in range(2):
                        for si, (n0, nw) in enumerate(nsubs):
                            ps[(m, si)] = psp.tile([128, SUB], dt.float32,
                                                   name=f"ps_{g0}_{m}_{si}", tag="ps")
                    for k in range(KCH):
                        kp = min(128, EDGES - k * 128)
                        umt = ump.tile([128, GROUP], dt.float8e4,
                                       name=f"um_{g0}_{k}", tag="um")
                        # alternate HWDGE queue families (SP/ACT) for the
                        # input stream: ~7us, same-window A/B verified
                        ieng = nc.scalar if k % 2 else nc.sync
                        ieng.dma_start(umt[:kp, :gw],
                                       um[k * 128:k * 128 + kp, g0:g0 + gw])
                        for m in range(2):
                            for si, (n0, nw) in enumerate(nsubs):
                                nc.tensor.matmul(
                                    ps[(m, si)][:, :nw],
                                    lhsT=ft_tiles[k][:kp, m * 128:(m + 1) * 128],
                                    rhs=umt[:kp, n0:n0 + nw],
                                    start=(k == 0),
                                    stop=(k == KCH - 1),
                                )
                    for m in range(2):
                        for si, (n0, nw) in enumerate(nsubs):
                            ot = obp.tile([128, SUB], dt.float32,
                                          name=f"ot_{g0}_{m}_{si}", tag="ot")
                            nc.vector.tensor_mul(ot[:, :nw], ps[(m, si)][:, :nw],
                                                 inv_sb[:, g0 + n0:g0 + n0 + nw])
                            # out-DMA via SWDGE: keeps the sync-engine HWDGE
                            # queues free for the um stream (~5us, measured)
                            nc.gpsimd.dma_start(out[m * 128:(m + 1) * 128,
                                                    g0 + n0:g0 + n0 + nw],
                                                ot[:, :nw])

            if reps == 1:
                body()
            else:
                with tc.For_i(0, reps, 1,
                              hint_engines=(mybir.EngineType.PE,
                                            mybir.EngineType.SP)):
                    body()
    nc.compile()
    return nc


def kernel(features, unroll_mat, occurrences):
    global _last_results
    features = np.asarray(features, dtype=np.float32)
    unroll_mat = np.asarray(unroll_mat, dtype=np.float32)
    occurrences = np.asarray(occurrences, dtype=np.float32)

    if "nc" not in _CACHE:
        _CACHE["nc"] = _build()
    nc = _CACHE["nc"]

    inv_full = (1.0 / occurrences).astype(np.float32)  # [B, TARGET]
    in_maps = []
    for c in range(NCORES):
        b, h = divmod(c, 2)
        fT = np.ascontiguousarray(features[b].T).astype(np.float16)
        um = np.ascontiguousarray(
            unroll_mat[b, :, h * COLS:(h + 1) * COLS]).astype(ml_dtypes.float8_e4m3)
        iv = np.ascontiguousarray(
            np.broadcast_to(inv_full[b, h * COLS:(h + 1) * COLS], (128, COLS)))
        in_maps.append({"fT": fT, "um": um, "inv": iv})

    res = run_bass_kernel_spmd(nc, in_maps, list(range(NCORES)))
    _last_results = res

    out = np.empty((B, NF, TARGET), dtype=np.float32)
    for c in range(NCORES):
        b, h = divmod(c, 2)
        out[b, :, h * COLS:(h + 1) * COLS] = res.results[c]["out"]
    return out

